# revision 1
# baseline (speedup 1.0000x reference)
"""Trainium2 Bass kernel for nn_BasicBlock (Minkowski sparse-conv basic block).

Strategy (8 NeuronCores, SPMD):
- Points N=400000 dest-sharded: core c owns output rows [c*50000,(c+1)*50000).
- Host routes messages: for each conv, msgs (k,e) grouped per (core, window of
  128 output rows, k). Each window has a uniform layout: 27 k-runs of 64 lanes
  (overflow msgs are pre-multiplied by W_k on host and placed in an identity-
  weight spill region), so one SPMD program serves all cores.
- Device per window: stream tile in (bf16) -> PE pack-2 transpose -> per-k
  matmul vs W_k (R-form, 32-aligned runs) -> one-hot P (is_equal vs iota) ->
  scatter matmul P^T @ msg accumulating the [128,64] window in PSUM -> flush.
- Instance-norm stats via ones-vector matmuls; AllReduce across cores for
  norm2; norm1 applied host-side when building the conv2 stream (h = relu(
  a1*y1+b1) feeds conv2's gather). Residual + final relu fused on device.
"""
import numpy as np
import ml_dtypes

N, C = 400000, 64
K, E = 27, 200000
EPS = 1e-5
NCORES = 8
SHARD = N // NCORES          # 50000
WIN = 128
NW = (SHARD + WIN - 1) // WIN  # 391
PADROWS = NW * WIN             # 50048
R = 64                         # lanes per k-run
BASE_LANES = K * R             # 1728

BF16 = ml_dtypes.bfloat16

_cache = {}


def _route_conv(out_idx_flat, k_flat):
    """Per (core, window, k) routing. Returns per-core dicts of lane tables."""
    core = out_idx_flat // SHARD
    rowpos = out_idx_flat - core * SHARD
    win = rowpos // WIN
    loc = rowpos - win * WIN
    return core, win, loc


def _build_stream(src_rows, core, win, loc, k_flat, Wk, n_blocks_min=14):
    """Build per-core lane tables.

    src_rows: [M,64] float32 message *inputs* (rows to be multiplied by W_k),
    core/win/loc/k_flat: [M] routing. Wk: [K,64,64].
    Returns list per core: (stream [128, B*NW*64] bf16, oiT [128, B*NW] bf16, B)
    Lane layout per window: lanes [k*64,(k+1)*64) = first 64 msgs of k;
    overflow lanes at [BASE_LANES, B*128) hold host-premultiplied msgs.
    """
    M = src_rows.shape[0]
    # rank of each msg within its (core,win,k) cell
    cell = (core.astype(np.int64) * NW + win) * K + k_flat
    order = np.argsort(cell, kind="stable")
    cell_s = cell[order]
    # rank within cell
    uniq, starts = np.unique(cell_s, return_index=True)
    rank = np.arange(M, dtype=np.int64)
    rank -= np.repeat(starts, np.diff(np.append(starts, M)))
    inrun = rank < R
    # spill ranks per (core,win)
    cw_s = cell_s // K
    spill_mask = ~inrun
    spill_cw = cw_s[spill_mask]
    so = np.argsort(spill_cw, kind="stable")
    su, sstarts = np.unique(spill_cw[so], return_index=True)
    srank = np.arange(spill_cw.shape[0], dtype=np.int64)
    srank -= np.repeat(sstarts, np.diff(np.append(sstarts, spill_cw.shape[0])))
    # spill capacity -> B
    max_spill = int(srank.max()) + 1 if srank.size else 0
    B = max(n_blocks_min, (BASE_LANES + max_spill + 127) // 128)
    LPW = B * 128
    # lane index within window
    lane = np.where(inrun, (cell_s % K) * R + rank, 0)
    lane_sp = np.zeros(M, np.int64)
    lane_sp_vals = BASE_LANES + srank
    tmp = np.zeros(spill_cw.shape[0], np.int64)
    tmp[so] = lane_sp_vals
    lane_sp[spill_mask] = tmp
    lane = np.where(inrun, lane, lane_sp)
    assert lane.max() < LPW

    rows = src_rows[order].astype(np.float32)
    kk = k_flat[order]
    # premultiply spill rows by their W_k
    if spill_mask.any():
        sm = spill_mask
        rows[sm] = np.einsum("mc,mcd->md", rows[sm], Wk[kk[sm]]).astype(np.float32)

    cores_out = []
    core_s = cw_s // NW
    win_s = cw_s % NW
    loc_s = loc[order]
    for c in range(NCORES):
        m = core_s == c
        gl = win_s[m] * LPW + lane[m]           # global lane in this core
        stream = np.zeros((NW * LPW, C), np.float32)
        stream[gl] = rows[m]
        oi = np.full(NW * LPW, -1.0, np.float32)
        oi[gl] = loc_s[m].astype(np.float32)
        # lane-major layouts: [128, nblk*64] and [128, nblk]
        nblk = NW * B
        stream_lm = np.ascontiguousarray(
            stream.reshape(nblk, 128, C).transpose(1, 0, 2).reshape(128, nblk * C)
        ).astype(BF16)
        oi_lm = np.ascontiguousarray(
            oi.reshape(nblk, 128).transpose(1, 0)
        ).astype(np.float32)
        cores_out.append((stream_lm, oi_lm))
    return cores_out, B


def _w_table(Wk):
    """[128, 28*64] f32: rows 0-63 and 64-127 both hold [W_0..W_26, I]."""
    wt = np.zeros((128, (K + 1) * C), np.float32)
    flat = np.concatenate([Wk.transpose(0, 1, 2).reshape(K * C, C),
                           np.eye(C, dtype=np.float32)], axis=0)  # [(K+1)*64,64]
    w2 = flat.reshape(K + 1, C, C)
    for k in range(K + 1):
        wt[0:64, k * C:(k + 1) * C] = w2[k]
        wt[64:128, k * C:(k + 1) * C] = w2[k]
    return wt


def _build_program(B, NW, NTOT, with_norm_out, ncores=8):
    """Build the SPMD Bass program for one conv.

    B: blocks per window; NW: windows; NTOT: total rows for mean divisor.
    """
    from concourse import bass, bacc, tile, mybir
    from concourse.masks import make_identity

    F32 = mybir.dt.float32
    BF = mybir.dt.bfloat16
    ActF = mybir.ActivationFunctionType
    Alu = mybir.AluOpType

    nc = bacc.Bacc("TRN2", target_bir_lowering=False, debug=False,
                   num_devices=ncores)
    stream_d = nc.dram_tensor("stream", [128, NW * B * C], BF,
                              kind="ExternalInput")
    oi_d = nc.dram_tensor("oiT", [128, NW * B], F32, kind="ExternalInput")
    wt_d = nc.dram_tensor("wt", [128, (K + 1) * C], F32, kind="ExternalInput")
    iota_d = nc.dram_tensor("iota", [128, 128], BF, kind="ExternalInput")
    if with_norm_out:
        xr_d = nc.dram_tensor("xr", [128, NW * C], F32, kind="ExternalInput")
        gb_d = nc.dram_tensor("gb", [1, 4 * C], F32, kind="ExternalInput")
        out_d = nc.dram_tensor("out", [128, NW * C], F32, kind="ExternalOutput")
    y_d = nc.dram_tensor("y", [128, NW * C], F32, kind="ExternalOutput")
    stats_d = nc.dram_tensor("stats", [1, 2 * C], F32, kind="ExternalOutput")

    STAGE = int(__import__("os").environ.get("KSTAGE", "9"))
    NPACK = (B * C + 127) // 128      # pack-2 transposes per window
    NPT = (NPACK + 3) // 4            # psum transpose tiles [128,512]
    NMT = (B + 7) // 8                # msg psum tiles [128,512] (8 blocks each)

    with tile.TileContext(nc) as tc:
        with (
            tc.tile_pool(name="const", bufs=1) as constp,
            tc.tile_pool(name="sb", bufs=3) as sb,
            tc.tile_pool(name="msb", bufs=2) as msb,
            tc.tile_pool(name="tp", bufs=2, space="PSUM") as tpp,
            tc.tile_pool(name="mp", bufs=1, space="PSUM") as mpp,
            tc.tile_pool(name="yp", bufs=2, space="PSUM") as ypp,
            tc.tile_pool(name="statp", bufs=1, space="PSUM") as statp,
            tc.tile_pool(name="dram", bufs=1, space="DRAM") as dramp,
        ):
            identb = constp.tile([128, 128], BF)
            make_identity(nc, identb[:])
            iota_t = constp.tile([128, 128], BF)
            nc.sync.dma_start(iota_t[:], iota_d[:])
            w_t = constp.tile([128, (K + 1) * C], F32)
            nc.sync.dma_start(w_t[:], wt_d[:])
            wb_t = constp.tile([128, (K + 1) * C], BF)
            nc.vector.tensor_copy(wb_t[:], w_t[:])
            ones_col = constp.tile([128, 1], F32)
            nc.gpsimd.memset(ones_col[:], 1.0)

            stat_sum = statp.tile([1, C], F32, tag="ssum")
            stat_sq = statp.tile([1, C], F32, tag="ssq")

            for s in range(NW):
                st = sb.tile([128, B * C], BF, tag="stream")
                nc.sync.dma_start(st[:], stream_d[:, s * B * C:(s + 1) * B * C])
                oi_t = sb.tile([128, B], F32, tag="oi")
                nc.sync.dma_start(oi_t[:], oi_d[:, s * B:(s + 1) * B])

                # per-block transposes (channels at rows 0-63) into
                # [64,512] psum tiles (4 blocks each), copy to SBUF
                xgT = sb.tile([64, B * 128], BF, tag="xgT")
                NTT = (B + 3) // 4
                for pt in range(NTT):
                    lo_b = pt * 4
                    hi_b = min(B, lo_b + 4)
                    tps = tpp.tile([64, 512], BF, tag="tps")
                    for b in range(lo_b, hi_b):
                        nc.tensor.transpose(
                            out=tps[0:64, (b - lo_b) * 128:(b - lo_b) * 128 + 128],
                            in_=st[:, b * C:(b + 1) * C],
                            identity=identb[:],
                        )
                    cw = (hi_b - lo_b) * 128
                    dst = xgT[:, lo_b * 128:lo_b * 128 + cw]
                    if pt % 2 == 0:
                        nc.scalar.activation(dst, tps[:, 0:cw], ActF.Copy)
                    else:
                        nc.vector.tensor_copy(dst, tps[:, 0:cw])

                # mm1: 27 k-runs of 64 lanes + identity spill runs
                msgps = []
                for j in range(NMT):
                    mpt = mpp.tile([128, 512], F32, tag=f"mps{j}", name=f"mps{j}")
                    msgps.append(mpt)

                def mm1(lane0, cnt, wslice):
                    j = lane0 // 128
                    lo = lane0 % 128
                    nc.tensor.matmul(
                        out=msgps[j // 8][lo:lo + cnt, (j % 8) * C:(j % 8 + 1) * C],
                        lhsT=xgT[0:64, j * 128 + lo:j * 128 + lo + cnt],
                        rhs=wb_t[0:64, wslice * C:(wslice + 1) * C],
                        start=True, stop=True,
                        tile_position=(0, lo),
                    )

                for k in range(K if STAGE >= 2 else 0):
                    mm1(k * 64, 64, k)
                a = BASE_LANES if STAGE >= 2 else B * 128
                while a < B * 128:
                    blk, lo = a // 128, a % 128
                    cap = {0: 128, 32: 32, 64: 64, 96: 32}[lo]
                    e = min(B * 128, blk * 128 + lo + cap)
                    mm1(a, e - a, K)
                    a = e

                msg = msb.tile([128, B * C], BF, tag="msg")
                for j in range(NMT):
                    w = min(512, (B - j * 8) * C)
                    dst = msg[:, j * 512:j * 512 + w]
                    if j % 2 == 0:
                        nc.vector.tensor_copy(dst, msgps[j][:, 0:w])
                    else:
                        nc.scalar.activation(dst, msgps[j][:, 0:w], ActF.Copy)

                # P-gen + scatter matmul into the window accumulator
                ywin = ypp.tile([WIN, C], F32, tag="ywin")
                for b in range(B if STAGE >= 3 else 1):
                    P = sb.tile([128, WIN], BF, tag="P")
                    eng = nc.vector if b % 2 == 0 else nc.gpsimd
                    eng.tensor_scalar(
                        out=P[:], in0=iota_t[:], scalar1=oi_t[:, b:b + 1],
                        scalar2=None, op0=Alu.is_equal,
                    )
                    nc.tensor.matmul(
                        out=ywin[:], lhsT=P[:], rhs=msg[:, b * C:(b + 1) * C],
                        start=(b == 0), stop=(b == B - 1) or STAGE < 3,
                    )

                yst = msb.tile([WIN, C], F32, tag="yst")
                nc.scalar.activation(yst[:], ywin[:], ActF.Copy)
                nc.sync.dma_start(y_d[:, s * C:(s + 1) * C], yst[:])
                ysq = msb.tile([WIN, C], F32, tag="ysq")
                nc.vector.tensor_tensor(out=ysq[:], in0=yst[:], in1=yst[:],
                                        op=Alu.mult)
                if STAGE >= 4:
                    nc.tensor.matmul(out=stat_sum[:], lhsT=ones_col[:],
                                     rhs=yst[:], start=(s == 0), stop=(s == NW - 1))
                    nc.tensor.matmul(out=stat_sq[:], lhsT=ones_col[:],
                                     rhs=ysq[:], start=(s == 0), stop=(s == NW - 1))

            stat_sb = sb.tile([1, 2 * C], F32, tag="statsb")
            nc.vector.tensor_copy(stat_sb[:, 0:C], stat_sum[:])
            nc.vector.tensor_copy(stat_sb[:, C:2 * C], stat_sq[:])
            nc.sync.dma_start(stats_d[:], stat_sb[:])

            if with_norm_out:
                b_in = dramp.tile([1, 2 * C], F32)
                b_out = dramp.tile([1, 2 * C], F32)
                nc.sync.dma_start(b_in[:], stat_sb[:])
                nc.gpsimd.collective_compute(
                    "AllReduce", Alu.add,
                    replica_groups=[list(range(ncores))],
                    ins=[b_in[:]], outs=[b_out[:]],
                )
                sall = sb.tile([1, 2 * C], F32, tag="sall")
                nc.sync.dma_start(sall[:], b_out[:])
                gbt = sb.tile([1, 4 * C], F32, tag="gbt")
                nc.sync.dma_start(gbt[:], gb_d[:])
                invN = 1.0 / float(NTOT)
                mu = sb.tile([1, C], F32, tag="mu")
                nc.vector.tensor_scalar(out=mu[:], in0=sall[0:1, 0:C],
                                        scalar1=invN, scalar2=None, op0=Alu.mult)
                ex2 = sb.tile([1, C], F32, tag="ex2")
                nc.vector.tensor_scalar(out=ex2[:], in0=sall[0:1, C:2 * C],
                                        scalar1=invN, scalar2=None, op0=Alu.mult)
                musq = sb.tile([1, C], F32, tag="musq")
                nc.vector.tensor_tensor(out=musq[:], in0=mu[:], in1=mu[:],
                                        op=Alu.mult)
                var = sb.tile([1, C], F32, tag="var")
                nc.vector.tensor_tensor(out=var[:], in0=ex2[:], in1=musq[:],
                                        op=Alu.subtract)
                epst = sb.tile([1, 1], F32, tag="epst")
                nc.gpsimd.memset(epst[:], EPS)
                vare = sb.tile([1, C], F32, tag="vare")
                nc.vector.tensor_scalar(out=vare[:], in0=var[:],
                                        scalar1=epst[0:1, 0:1], scalar2=None,
                                        op0=Alu.add)
                sd = sb.tile([1, C], F32, tag="sd")
                nc.scalar.activation(sd[:], vare[:], ActF.Sqrt)
                rstd = sb.tile([1, C], F32, tag="rstd")
                nc.vector.reciprocal(rstd[:], sd[:])
                a_c = sb.tile([1, C], F32, tag="a_c")
                nc.vector.tensor_tensor(out=a_c[:], in0=rstd[:],
                                        in1=gbt[0:1, 0:C], op=Alu.mult)
                mua = sb.tile([1, C], F32, tag="mua")
                nc.vector.tensor_tensor(out=mua[:], in0=mu[:], in1=a_c[:],
                                        op=Alu.mult)
                b_c = sb.tile([1, C], F32, tag="b_c")
                nc.vector.tensor_tensor(out=b_c[:], in0=gbt[0:1, C:2 * C],
                                        in1=mua[:], op=Alu.subtract)
                ones_row = constp.tile([1, 128], F32)
                nc.gpsimd.memset(ones_row[:], 1.0)
                a_rep = constp.tile([128, C], F32)
                b_rep = constp.tile([128, C], F32)
                abp = ypp.tile([128, C], F32, tag="ywin")
                nc.tensor.matmul(out=abp[:], lhsT=ones_row[:], rhs=a_c[:],
                                 start=True, stop=True)
                nc.scalar.activation(a_rep[:], abp[:], ActF.Copy)
                abp2 = ypp.tile([128, C], F32, tag="ywin")
                nc.tensor.matmul(out=abp2[:], lhsT=ones_row[:], rhs=b_c[:],
                                 start=True, stop=True)
                nc.scalar.activation(b_rep[:], abp2[:], ActF.Copy)

                for s in range(NW):
                    yt = sb.tile([128, C], F32, tag="yt")
                    nc.sync.dma_start(yt[:], y_d[:, s * C:(s + 1) * C])
                    xrt = sb.tile([128, C], F32, tag="xrt")
                    nc.sync.dma_start(xrt[:], xr_d[:, s * C:(s + 1) * C])
                    t1 = sb.tile([128, C], F32, tag="t1")
                    nc.vector.tensor_tensor(out=t1[:], in0=yt[:], in1=a_rep[:],
                                            op=Alu.mult)
                    t2 = sb.tile([128, C], F32, tag="t2")
                    nc.vector.tensor_tensor(out=t2[:], in0=t1[:], in1=b_rep[:],
                                            op=Alu.add)
                    t3 = sb.tile([128, C], F32, tag="t3")
                    nc.vector.tensor_tensor(out=t3[:], in0=t2[:], in1=xrt[:],
                                            op=Alu.add)
                    t4 = sb.tile([128, C], F32, tag="t4")
                    nc.scalar.activation(t4[:], t3[:], ActF.Relu)
                    nc.sync.dma_start(out_d[:, s * C:(s + 1) * C], t4[:])

    nc.compile()
    return nc


def _lane_major_rows(arr_lm, nblk):
    """[128, nblk*64] lane-major -> [nblk*128, 64] rows."""
    return arr_lm.reshape(128, nblk, C).transpose(1, 0, 2).reshape(nblk * 128, C)


def _rows_to_lane_major(rows, nblk):
    return np.ascontiguousarray(
        rows.reshape(nblk, 128, C).transpose(1, 0, 2).reshape(128, nblk * C))


def kernel(x, in_idx, out_idx, W1, W2, gamma1, beta1, gamma2, beta2,
           profile=False):
    from concourse.bass_utils import run_bass_kernel_spmd

    x = np.asarray(x, np.float32)
    in_idx = np.asarray(in_idx)
    out_idx = np.asarray(out_idx)
    W1 = np.asarray(W1, np.float32)
    W2 = np.asarray(W2, np.float32)
    g1 = np.asarray(gamma1, np.float32)
    b1 = np.asarray(beta1, np.float32)
    g2 = np.asarray(gamma2, np.float32)
    b2 = np.asarray(beta2, np.float32)

    ii = in_idx.reshape(-1).astype(np.int64)
    oo = out_idx.reshape(-1).astype(np.int64)
    kf = np.repeat(np.arange(K, dtype=np.int64), in_idx.shape[1])
    core, win, loc = _route_conv(oo, kf)

    iota = np.broadcast_to(np.arange(128, dtype=np.float32),
                           (128, 128)).astype(BF16).copy()

    # ---- conv1 ----
    rows1 = x[ii]
    cores1, B1 = _build_stream(rows1, core, win, loc, kf, W1)
    wt1 = _w_table(W1)
    prog_key = ("A", B1, NW)
    if prog_key not in _cache:
        _cache[prog_key] = _build_program(B1, NW, N, with_norm_out=False)
    ncA = _cache[prog_key]
    in_maps = [{"stream": s, "oiT": o, "wt": wt1, "iota": iota}
               for (s, o) in cores1]
    import time as _t
    _t0 = _t.time()
    resA = run_bass_kernel_spmd(ncA, in_maps, core_ids=list(range(NCORES)),
                                trace=profile)
    kernel._runA_s = _t.time() - _t0
    y1 = np.zeros((NCORES * PADROWS, C), np.float32)
    stats1 = np.zeros((2, C), np.float64)
    for c in range(NCORES):
        y1[c * PADROWS:(c + 1) * PADROWS] = _lane_major_rows(
            resA.results[c]["y"], NW)
        stats1 += resA.results[c]["stats"].reshape(2, C).astype(np.float64)

    # norm1 on host (builds conv2 stream input h)
    mu1 = (stats1[0] / N).astype(np.float32)
    var1 = (stats1[1] / N).astype(np.float32) - mu1 * mu1
    a1 = (g1 / np.sqrt(var1 + EPS)).astype(np.float32)
    bb1 = (b1 - mu1 * a1).astype(np.float32)
    # gather h rows for conv2: h[i] = relu(a1*y1[i] + bb1)
    ii_pad = (ii // SHARD) * PADROWS + (ii % SHARD)
    rows2 = np.maximum(y1[ii_pad] * a1 + bb1, 0.0)

    # ---- conv2 ----
    cores2, B2 = _build_stream(rows2, core, win, loc, kf, W2)
    wt2 = _w_table(W2)
    prog_key2 = ("B", B2, NW)
    if prog_key2 not in _cache:
        _cache[prog_key2] = _build_program(B2, NW, N, with_norm_out=True)
    ncB = _cache[prog_key2]
    gb = np.concatenate([g2, b2, np.zeros_like(g2), np.zeros_like(b2)])[None, :]
    in_maps2 = []
    for c in range(NCORES):
        xr = np.zeros((PADROWS, C), np.float32)
        xr[:SHARD] = x[c * SHARD:(c + 1) * SHARD]
        in_maps2.append({
            "stream": cores2[c][0], "oiT": cores2[c][1], "wt": wt2,
            "iota": iota, "xr": _rows_to_lane_major(xr, NW), "gb": gb,
        })
    _t0 = _t.time()
    resB = run_bass_kernel_spmd(ncB, in_maps2, core_ids=list(range(NCORES)),
                                trace=profile)
    kernel._runB_s = _t.time() - _t0
    out = np.zeros((N, C), np.float32)
    for c in range(NCORES):
        rows = _lane_major_rows(resB.results[c]["out"], NW)
        out[c * SHARD:(c + 1) * SHARD] = rows[:SHARD]
    kernel._last = (resA, resB)
    return out



# revision 3
# speedup vs baseline: 56.7872x; 56.7872x over previous
"""Trainium2 Bass kernel for nn_BasicBlock (Minkowski sparse-conv block).

Single fused SPMD program on 8 cores, dest-sharded (core c owns output rows
[c*SHARD, (c+1)*SHARD)):
  AllGather x shards -> x_gat [8*PADSH, 128] bf16 in HBM
  conv: SWDGE dma_gather (transpose mode -> channel-major) per (d,k,s) run
        -> per-128-lane matmul vs W_k -> PSUM -> SBUF f32
        -> SWDGE dma_scatter_add into local y [ND*CH, 64] f32 (runtime counts)
  stats via ones-matmul + AllReduce, per-channel affine applied in flat
  tiles; h written bf16 padded-128, AllGather -> conv2 -> norm2 + residual
  + relu -> out shard bf16.

Host ships only: x shard bf16, compact wrapped idx tables (shared by both
convs), per-run counts, weights, gamma/beta.
"""
import numpy as np
import ml_dtypes

BF16 = ml_dtypes.bfloat16
EPS = 1e-5


class CFG:
    N, C = 400000, 64
    K, E = 27, 200000
    NCORES = 8
    SHARD = 50000
    PADSH = 50048              # padded shard rows (multiple of 128)
    CH = 32768                 # int16 index chunk
    SB_LANES = 8192            # lanes per superblock
    ROWT = 2048                # rows per flat norm tile

    @classmethod
    def derived(cls):
        cls.ND = (cls.SHARD + cls.CH - 1) // cls.CH
        cls.NS = (cls.NCORES * cls.PADSH + cls.CH - 1) // cls.CH
        cls.YROWS = cls.ND * cls.CH
        cls.GROWS = cls.NS * cls.CH
        cls.NT_FULL = cls.PADSH // cls.ROWT
        cls.TAILR = cls.PADSH - cls.NT_FULL * cls.ROWT
        assert cls.TAILR % 128 == 0 and cls.SHARD <= cls.PADSH
        assert cls.NCORES * cls.PADSH <= cls.GROWS


CFG.derived()

_cache = {}


def _route(in_idx, out_idx):
    """Host routing with per-run occurrence rounds (sub-runs).

    Within one dma_scatter_add the destination rows must be unique (the
    DMA's read-modify-write races otherwise), so each (d,k,s) run is split
    into sub-runs: sub-run r holds the r-th occurrence of each destination
    within that (core,run). Sub-runs become separate scatter instructions
    (tile serializes same-chunk scatters).

    Returns (runs, TOT, gidx, sidx, cnt_sub) where each run is
    (off, Ltot, d, k, s, [(sub_off, sub_L, cnt_col), ...]).
    """
    c = CFG
    ii = in_idx.reshape(-1).astype(np.int64)
    oo = out_idx.reshape(-1).astype(np.int64)
    M = ii.shape[0]
    kf = np.repeat(np.arange(c.K, dtype=np.int64), in_idx.shape[1])

    core = oo // c.SHARD
    dl = oo - core * c.SHARD
    d = dl // c.CH
    ld = dl - d * c.CH                       # scatter idx within chunk
    cs = ii // c.SHARD
    g = cs * c.PADSH + (ii - cs * c.SHARD)   # row in x_gat
    s = g // c.CH
    li = g - s * c.CH                        # gather idx within chunk

    NRALL = c.ND * c.K * c.NS
    rkey = (d * c.K + kf) * c.NS + s
    ckey = core * NRALL + rkey               # (core, run)

    # occurrence rank of each msg within (core, run, dst)
    okey = ckey * c.CH + ld
    oorder = np.argsort(okey, kind="stable")
    okey_s = okey[oorder]
    brk = np.r_[0, np.nonzero(np.diff(okey_s))[0] + 1]
    glen = np.diff(np.r_[brk, M])
    occ_s = np.arange(M, dtype=np.int64) - np.repeat(brk, glen)
    mult_s = np.repeat(glen, glen)
    occ = np.empty(M, np.int64)
    occ[oorder] = occ_s
    mult = np.empty(M, np.int64)
    mult[oorder] = mult_s

    # counts per (core, run, round)
    R = int(occ.max()) + 1
    cnt3 = np.bincount((ckey * R + occ).astype(np.int64),
                       minlength=c.NCORES * NRALL * R).reshape(
        c.NCORES, NRALL, R)
    cnt_r = cnt3.sum(0).astype(np.int64)          # total per (run, round)
    run_mask = cnt_r[:, 0] > 0
    maxr = (cnt_r > 0).argmin(1)                  # rounds per run
    maxr[cnt_r[:, -1] > 0] = R
    maxr[~run_mask] = 0

    # borrow: any (core, run, round<maxr) with 0 count gets one singleton
    # (mult==1, occ==0) lane moved into that round
    need = (cnt3 == 0) & (np.arange(R)[None, None, :] < maxr[None, :, None])
    if need.any():
        nc_, nr_, nro_ = np.nonzero(need)
        for cc, rr, ro in zip(nc_, nr_, nro_):
            cand = np.nonzero((ckey == cc * NRALL + rr) & (occ == 0)
                              & (mult == 1))[0]
            assert cand.size > 0, "no singleton to borrow"
            occ[cand[0]] = ro
            mult[cand[0]] = 0                     # not reusable
        cnt3 = np.bincount((ckey * R + occ).astype(np.int64),
                           minlength=c.NCORES * NRALL * R).reshape(
            c.NCORES, NRALL, R)

    # sub-run padded lengths and offsets
    Lsub = ((cnt3.max(0) + 127) // 128) * 128     # [NRALL, R]
    Ltot = Lsub.sum(1)
    run_ids = np.nonzero(Ltot)[0]
    roff = np.zeros(NRALL, np.int64)
    roff[run_ids] = np.cumsum(Ltot[run_ids]) - Ltot[run_ids]
    soff = np.cumsum(Lsub, 1) - Lsub              # sub offsets within run
    TOT = int(Ltot.sum())

    # lane position: sort by (core, run, round), rank within group
    skey = ckey * R + occ
    sorder = np.argsort(skey, kind="stable")
    skey_s = skey[sorder]
    sbrk = np.r_[0, np.nonzero(np.diff(skey_s))[0] + 1]
    sglen = np.diff(np.r_[sbrk, M])
    rank = np.arange(M, dtype=np.int64) - np.repeat(sbrk, sglen)
    rk_s = skey_s % (NRALL * R)
    run_s = rk_s // R
    rnd_s = rk_s % R
    lane = roff[run_s] + soff[run_s, rnd_s] + rank
    core_s = skey_s // (NRALL * R)

    gidx = np.zeros((c.NCORES, TOT), np.int16)
    sidx = np.full((c.NCORES, TOT), -1, np.int16)
    li_s = li[sorder]
    ld_s = ld[sorder]
    for cc in range(c.NCORES):
        m = core_s == cc
        gidx[cc, lane[m]] = li_s[m]
        sidx[cc, lane[m]] = ld_s[m]

    # per-core counts per sub-run (compacted column order)
    runs = []
    cols = []
    for r in run_ids:
        s_ = r % c.NS
        k_ = (r // c.NS) % c.K
        d_ = r // (c.NS * c.K)
        subs = []
        for ro in range(int(maxr[r])):
            subs.append((int(soff[r, ro]), int(Lsub[r, ro]), len(cols)))
            cols.append((r, ro))
        runs.append((int(roff[r]), int(Ltot[r]), int(d_), int(k_), int(s_),
                     subs))
    cnt_sub = np.stack([cnt3[:, r, ro] for (r, ro) in cols],
                       axis=1).astype(np.int32)
    assert (cnt_sub > 0).all(), "zero-count sub-run would hang scatter sem"
    return runs, TOT, gidx, sidx, cnt_sub


def _wrap16(a):
    """[..., n] -> [..., 16, n/16] wrapped: entry j at (j%16, j//16)."""
    n = a.shape[-1]
    assert n % 16 == 0
    if a.ndim == 2:
        return np.ascontiguousarray(a.reshape(a.shape[0], n // 16, 16)
                                    .transpose(0, 2, 1))
    return np.ascontiguousarray(a.reshape(n // 16, 16).T)


def _superblocks(runs):
    sbs, cur, acc = [], [], 0
    for r in runs:
        assert r[1] <= CFG.SB_LANES
        if acc + r[1] > CFG.SB_LANES and cur:
            sbs.append(cur)
            cur, acc = [], 0
        cur.append(r)
        acc += r[1]
    if cur:
        sbs.append(cur)
    return sbs


def _build_program(runs, TOT):
    from concourse import bacc, tile, mybir
    from concourse import library_config

    c = CFG
    F32 = mybir.dt.float32
    BF = mybir.dt.bfloat16
    I16 = mybir.dt.int16
    I32 = mybir.dt.int32
    ActF = mybir.ActivationFunctionType
    Alu = mybir.AluOpType

    NRUNS = sum(len(r[5]) for r in runs)     # scatter count columns
    sbs = _superblocks(runs)
    SBL = c.SB_LANES
    FTW = (c.ROWT // 128) * 64          # full row-tile width (1024)

    nc = bacc.Bacc("TRN2", target_bir_lowering=False, debug=False,
                   num_devices=c.NCORES)

    x_d = nc.dram_tensor("xs", [c.PADSH, 128], BF, kind="ExternalInput")
    gidx_d = nc.dram_tensor("gidx", [16, TOT // 16], I16,
                            kind="ExternalInput")
    sidx_d = nc.dram_tensor("sidx", [16, TOT // 16], I16,
                            kind="ExternalInput")
    cnt_d = nc.dram_tensor("cnt", [1, NRUNS], I32, kind="ExternalInput")
    w_d = nc.dram_tensor("wt", [64, 2 * c.K * 64], F32, kind="ExternalInput")
    gb_d = nc.dram_tensor("gb", [1, 4 * 64], F32, kind="ExternalInput")
    out_d = nc.dram_tensor("out", [c.PADSH, 64], BF, kind="ExternalOutput")

    with tile.TileContext(nc) as tc:
        with (
            tc.tile_pool(name="const", bufs=1) as constp,
            tc.tile_pool(name="gp", bufs=2) as gpool,
            tc.tile_pool(name="mp", bufs=2) as mpool,
            tc.tile_pool(name="ip", bufs=3) as ipool,
            tc.tile_pool(name="sp", bufs=2) as spool,
            tc.tile_pool(name="psmm", bufs=4, space="PSUM") as psmm,
            tc.tile_pool(name="psbc", bufs=1, space="PSUM") as psbc,
            tc.tile_pool(name="psst", bufs=1, space="PSUM") as psst,
            tc.tile_pool(name="dram", bufs=1, space="DRAM") as dramp,
        ):
            nc.gpsimd.load_library(library_config.mlp)

            # ---------- constants
            w_f = constp.tile([64, 2 * c.K * 64], F32)
            nc.sync.dma_start(w_f[:], w_d[:])
            w_b = constp.tile([64, 2 * c.K * 64], BF)
            nc.vector.tensor_copy(w_b[:], w_f[:])
            gb_t = constp.tile([1, 4 * 64], F32)
            nc.sync.dma_start(gb_t[:], gb_d[:])
            ones_col = constp.tile([128, 1], F32)
            nc.vector.memset(ones_col[:], 1.0)
            ones_row = constp.tile([1, 128], F32)
            nc.vector.memset(ones_row[:], 1.0)
            epst = constp.tile([1, 1], F32)
            nc.vector.memset(epst[:], EPS)
            zt = constp.tile([128, 4096], F32)
            nc.vector.memset(zt[:], 0.0)

            cnt_t = constp.tile([1, NRUNS], I32)
            nc.sync.dma_start(cnt_t[:], cnt_d[:])

            # ---------- DRAM scratch
            x_gat = dramp.tile([c.GROWS, 128], BF)
            h_loc = dramp.tile([c.PADSH, 128], BF)
            h_gat = dramp.tile([c.GROWS, 128], BF)
            y1 = dramp.tile([c.YROWS, 64], F32)
            y2 = dramp.tile([c.YROWS, 64], F32)
            st_in = dramp.tile([1, 128], F32)
            st_out = dramp.tile([1, 128], F32)

            for yb in (y1, y2):
                yv = yb[:].rearrange("(p b) c -> p (b c)", p=128)
                wv = yv.shape[1]
                for j in range(0, wv, 4096):
                    zw = min(4096, wv - j)
                    nc.sync.dma_start(yv[:, j:j + zw], zt[:, 0:zw])

            x_stage = dramp.tile([c.PADSH, 128], BF)
            nc.sync.dma_start(x_stage[:], x_d[:])
            nc.gpsimd.collective_compute(
                "AllGather", Alu.bypass,
                replica_groups=[list(range(c.NCORES))],
                ins=[x_stage[:]], outs=[x_gat[0:c.NCORES * c.PADSH, :]],
            )

            creg = nc.gpsimd.alloc_register("scnt")

            # ---------- sparse conv
            def issue_gathers(sb, src_gat):
                lanes = sum(r[1] for r in sb)
                base = sb[0][0]
                gi_t = ipool.tile([128, SBL // 16], I16, tag="gi")
                si_t = ipool.tile([128, SBL // 16], I16, tag="si")
                for (t, d_src) in ((gi_t, gidx_d), (si_t, sidx_d)):
                    nc.sync.dma_start(
                        t[0:16, 0:lanes // 16],
                        d_src[:, base // 16:(base + lanes) // 16])
                    nc.sync.dma_start(t[16:32, 0:lanes // 16],
                                      t[0:16, 0:lanes // 16])
                    nc.sync.dma_start(t[32:64, 0:lanes // 16],
                                      t[0:32, 0:lanes // 16])
                    nc.sync.dma_start(t[64:128, 0:lanes // 16],
                                      t[0:64, 0:lanes // 16])
                GCAP = 896   # transpose dma_gather hangs at >=1024 idxs
                g_t = gpool.tile([128, SBL], BF, tag="g")
                for (off, L, d_, k_, s_, subs) in sb:
                    lo = off - base
                    for p0 in range(0, L, GCAP):
                        pL = min(GCAP, L - p0)
                        a = lo + p0
                        nc.gpsimd.dma_gather(
                            out_ap=g_t[:, a:a + pL].unsqueeze(1),
                            in_ap=src_gat[s_ * c.CH:(s_ + 1) * c.CH, :],
                            idxs_ap=gi_t[:, a // 16:(a + pL) // 16],
                            num_idxs=pL,
                            num_idxs_reg=pL,
                            elem_size=128,
                            transpose=True,
                        )
                return g_t, si_t

            def compute_and_scatter(sb, g_t, si_t, y_dst, kofs):
                lanes = sum(r[1] for r in sb)
                nblk = lanes // 128
                base = sb[0][0]
                msg_t = mpool.tile([128, (SBL // 128) * 64], F32, tag="msg")
                bk = []
                for (off, L, d_, k_, s_, subs) in sb:
                    bk += [k_] * (L // 128)
                for g0 in range(0, nblk, 8):
                    g1 = min(nblk, g0 + 8)
                    mm = psmm.tile([128, 512], F32, tag="mm")
                    for b in range(g0, g1):
                        nc.tensor.matmul(
                            out=mm[:, (b - g0) * 64:(b - g0 + 1) * 64],
                            lhsT=g_t[0:64, b * 128:(b + 1) * 128],
                            rhs=w_b[:, (kofs + bk[b]) * 64:
                                    (kofs + bk[b] + 1) * 64],
                            start=True, stop=True,
                        )
                    dst = msg_t[:, g0 * 64:g1 * 64]
                    src = mm[:, 0:(g1 - g0) * 64]
                    if (g0 // 8) % 2 == 0:
                        nc.scalar.activation(dst, src, ActF.Copy)
                    else:
                        nc.vector.tensor_copy(dst, src)
                for (off, L, d_, k_, s_, subs) in sb:
                    lo = off - base
                    for (so, sL, col) in subs:
                        a = lo + so
                        nc.gpsimd.reg_load(creg, cnt_t[0:1, col:col + 1])
                        nc.gpsimd.dma_scatter_add(
                            out_ap=y_dst[d_ * c.CH:(d_ + 1) * c.CH, :],
                            in_ap=msg_t[:, a // 128 * 64:(a + sL) // 128 * 64]
                            .rearrange("p (b ch) -> p b ch", ch=64),
                            idxs_ap=si_t[:, a // 16:(a + sL) // 16],
                            num_idxs=sL,
                            num_idxs_reg=creg,
                            elem_size=64,
                        )

            def conv(src_gat, y_dst, kofs):
                stage = []
                for sb in sbs:
                    g_t, si_t = issue_gathers(sb, src_gat)
                    stage.append((sb, g_t, si_t))
                    if len(stage) >= 2:
                        psb, pg, psi = stage.pop(0)
                        compute_and_scatter(psb, pg, psi, y_dst, kofs)
                while stage:
                    psb, pg, psi = stage.pop(0)
                    compute_and_scatter(psb, pg, psi, y_dst, kofs)

            def row_tiles():
                out = []
                for t in range(c.NT_FULL + 1):
                    r0 = t * c.ROWT
                    nr = c.ROWT if t < c.NT_FULL else c.TAILR
                    if nr:
                        out.append((t, r0, nr, nr // 128))
                return out

            def load_rowtile(y_src, r0, nr, w, tag):
                yv = spool.tile([128, FTW], F32, tag=tag)
                if w < FTW:
                    nc.vector.memset(yv[:, 0:FTW], 0.0)
                nc.sync.dma_start(
                    yv[:, 0:w],
                    y_src[r0:r0 + nr, :].rearrange("(p b) ch -> p (b ch)",
                                                   p=128))
                return yv

            SW = min(512, FTW)          # stats psum width

            def stats_affine(y_src):
                """Channel sums + sumsq over the shard, AllReduced."""
                tiles = row_tiles()
                ssum = psst.tile([1, SW], F32, tag="ssum")
                ssq = psst.tile([1, SW], F32, tag="ssq")
                nchunk = len(tiles) * (FTW // SW)
                qi = 0
                for t, r0, nr, nb in tiles:
                    w = nb * 64
                    yv = load_rowtile(y_src, r0, nr, w, "yv")
                    sq = spool.tile([128, FTW], F32, tag="sq")
                    nc.scalar.activation(sq[:, 0:w], yv[:, 0:w], ActF.Square)
                    if w < FTW:
                        nc.vector.memset(sq[:, w:FTW], 0.0)
                    for c0 in range(0, FTW, SW):
                        nc.tensor.matmul(out=ssum[:], lhsT=ones_col[:],
                                         rhs=yv[:, c0:c0 + SW],
                                         start=(qi == 0),
                                         stop=(qi == nchunk - 1))
                        nc.tensor.matmul(out=ssq[:], lhsT=ones_col[:],
                                         rhs=sq[:, c0:c0 + SW],
                                         start=(qi == 0),
                                         stop=(qi == nchunk - 1))
                        qi += 1
                # fold SW columns down to 64, pack [sum, sumsq] into [1,128]
                fold = spool.tile([1, 2 * SW], F32, tag="fold")
                nc.vector.tensor_copy(fold[:, 0:SW], ssum[:])
                nc.vector.tensor_copy(fold[:, SW:2 * SW], ssq[:])
                for half in range(2):
                    b0 = half * SW
                    step = SW // 2
                    while step >= 64:
                        nc.vector.tensor_tensor(
                            out=fold[:, b0:b0 + step],
                            in0=fold[:, b0:b0 + step],
                            in1=fold[:, b0 + step:b0 + 2 * step],
                            op=Alu.add)
                        step //= 2
                packed = spool.tile([1, 128], F32, tag="packed")
                nc.vector.tensor_copy(packed[:, 0:64], fold[:, 0:64])
                nc.vector.tensor_copy(packed[:, 64:128], fold[:, SW:SW + 64])
                nc.sync.dma_start(st_in[:], packed[:])
                nc.gpsimd.collective_compute(
                    "AllReduce", Alu.add,
                    replica_groups=[list(range(c.NCORES))],
                    ins=[st_in[:]], outs=[st_out[:]],
                )
                allst = spool.tile([1, 128], F32, tag="allst")
                nc.sync.dma_start(allst[:], st_out[:])
                return allst

            def affine_consts(allst, gofs):
                """a = gamma*rsqrt(var+eps), b = beta - mu*a; [128,FTW] reps."""
                invN = 1.0 / float(c.N)
                mu = spool.tile([1, 64], F32, tag="mu")
                nc.vector.tensor_scalar(out=mu[:], in0=allst[0:1, 0:64],
                                        scalar1=invN, scalar2=None,
                                        op0=Alu.mult)
                ex2 = spool.tile([1, 64], F32, tag="ex2")
                nc.vector.tensor_scalar(out=ex2[:], in0=allst[0:1, 64:128],
                                        scalar1=invN, scalar2=None,
                                        op0=Alu.mult)
                var = spool.tile([1, 64], F32, tag="var")
                nc.vector.tensor_tensor(out=var[:], in0=mu[:], in1=mu[:],
                                        op=Alu.mult)
                nc.vector.tensor_tensor(out=var[:], in0=ex2[:], in1=var[:],
                                        op=Alu.subtract)
                nc.vector.tensor_scalar(out=var[:], in0=var[:],
                                        scalar1=epst[0:1, 0:1], scalar2=None,
                                        op0=Alu.add)
                sd = spool.tile([1, 64], F32, tag="sd")
                nc.scalar.activation(sd[:], var[:], ActF.Sqrt)
                rstd = spool.tile([1, 64], F32, tag="rstd")
                nc.vector.reciprocal(rstd[:], sd[:])
                a_c = spool.tile([1, 64], F32, tag="a_c")
                nc.vector.tensor_tensor(out=a_c[:], in0=rstd[:],
                                        in1=gb_t[0:1, gofs:gofs + 64],
                                        op=Alu.mult)
                b_c = spool.tile([1, 64], F32, tag="b_c")
                nc.vector.tensor_tensor(out=b_c[:], in0=mu[:], in1=a_c[:],
                                        op=Alu.mult)
                nc.vector.tensor_tensor(out=b_c[:],
                                        in0=gb_t[0:1, gofs + 64:gofs + 128],
                                        in1=b_c[:], op=Alu.subtract)
                # broadcast to 128 partitions, tile 16x along free
                reps = []
                for src in (a_c, b_c):
                    bc = psbc.tile([128, 64], F32, tag="bc")
                    nc.tensor.matmul(out=bc[:], lhsT=ones_row[:], rhs=src[:],
                                     start=True, stop=True)
                    rep = spool.tile([128, FTW], F32, tag=f"rep{len(reps)}")
                    nc.scalar.activation(rep[:, 0:64], bc[:], ActF.Copy)
                    width = 64
                    while width < FTW:
                        wnext = min(FTW, 2 * width)
                        nc.vector.tensor_copy(rep[:, width:wnext],
                                              rep[:, 0:wnext - width])
                        width = wnext
                    reps.append(rep)
                return reps

            def apply_norm(y_src, a_rep, b_rep, mode):
                """mode 'h': h_loc = relu(a*y+b) bf16 (cols 0:64).
                   mode 'out': out_d = relu(a*y+b + x) bf16."""
                for t, r0, nr, nb in row_tiles():
                    w = nb * 64
                    yv = load_rowtile(y_src, r0, nr, w, "ya")
                    nc.vector.tensor_tensor(out=yv[:, 0:w], in0=yv[:, 0:w],
                                            in1=a_rep[:, 0:w], op=Alu.mult)
                    nc.vector.tensor_tensor(out=yv[:, 0:w], in0=yv[:, 0:w],
                                            in1=b_rep[:, 0:w], op=Alu.add)
                    if mode == "out":
                        xb = spool.tile([128, FTW], BF, tag="xb")
                        nc.sync.dma_start(
                            xb[:, 0:w].rearrange("p (b ch) -> p b ch",
                                                 ch=64),
                            x_d[r0:r0 + nr, 0:64].rearrange(
                                "(p b) ch -> p b ch", p=128))
                        xf = spool.tile([128, FTW], F32, tag="xf")
                        nc.scalar.activation(xf[:, 0:w], xb[:, 0:w],
                                             ActF.Copy)
                        nc.vector.tensor_tensor(out=yv[:, 0:w],
                                                in0=yv[:, 0:w],
                                                in1=xf[:, 0:w], op=Alu.add)
                    ob = spool.tile([128, FTW], BF, tag="ob")
                    nc.scalar.activation(ob[:, 0:w], yv[:, 0:w], ActF.Relu)
                    if mode == "h":
                        nc.sync.dma_start(
                            h_loc[r0:r0 + nr, 0:64].rearrange(
                                "(p b) ch -> p b ch", p=128),
                            ob[:, 0:w].rearrange("p (b ch) -> p b ch",
                                                 ch=64))
                    else:
                        nc.sync.dma_start(
                            out_d[r0:r0 + nr, :].rearrange(
                                "(p b) ch -> p (b ch)", p=128),
                            ob[:, 0:w])

            # ---------------- pipeline
            conv(x_gat, y1, kofs=0)
            allst1 = stats_affine(y1)
            a1, b1 = affine_consts(allst1, gofs=0)
            apply_norm(y1, a1, b1, "h")
            nc.gpsimd.collective_compute(
                "AllGather", Alu.bypass,
                replica_groups=[list(range(c.NCORES))],
                ins=[h_loc[:]], outs=[h_gat[0:c.NCORES * c.PADSH, :]],
            )
            conv(h_gat, y2, kofs=c.K)
            allst2 = stats_affine(y2)
            a2, b2 = affine_consts(allst2, gofs=128)
            apply_norm(y2, a2, b2, "out")

    nc.compile()
    return nc


def kernel(x, in_idx, out_idx, W1, W2, gamma1, beta1, gamma2, beta2,
           profile=False):
    from concourse.bass_utils import run_bass_kernel_spmd
    import time as _t

    c = CFG
    x = np.asarray(x, np.float32)
    W1 = np.asarray(W1, np.float32)
    W2 = np.asarray(W2, np.float32)
    g1 = np.asarray(gamma1, np.float32)
    b1 = np.asarray(beta1, np.float32)
    g2 = np.asarray(gamma2, np.float32)
    b2 = np.asarray(beta2, np.float32)

    runs, TOT, gidx, sidx, cnt_sub = _route(np.asarray(in_idx),
                                            np.asarray(out_idx))

    key = (TOT, tuple((r[0], r[1], tuple(r[5])) for r in runs))
    if key not in _cache:
        _cache[key] = _build_program(runs, TOT)
    nc = _cache[key]

    # W layout: [64 cin, (conv,k) * 64 cout]
    wt = np.concatenate([W1, W2], axis=0).transpose(1, 0, 2).reshape(
        64, 2 * c.K * 64)
    wt = np.ascontiguousarray(wt)
    gb = np.concatenate([g1, b1, g2, b2])[None, :].astype(np.float32)

    in_maps = []
    for cc in range(c.NCORES):
        xs = np.zeros((c.PADSH, 128), BF16)
        xs[0:c.SHARD, 0:64] = x[cc * c.SHARD:(cc + 1) * c.SHARD]
        in_maps.append({
            "xs": xs,
            "gidx": _wrap16(gidx[cc]),
            "sidx": _wrap16(sidx[cc]),
            "cnt": cnt_sub[cc][None, :],
            "wt": wt,
            "gb": gb,
        })

    t0 = _t.time()
    res = run_bass_kernel_spmd(nc, in_maps, core_ids=list(range(c.NCORES)),
                               trace=profile)
    kernel._run_s = _t.time() - t0
    kernel._last = res

    out = np.empty((c.N, c.C), np.float32)
    for cc in range(c.NCORES):
        out[cc * c.SHARD:(cc + 1) * c.SHARD] = \
            res.results[cc]["out"][0:c.SHARD].astype(np.float32)
    return out


# revision 4
# speedup vs baseline: 110.9100x; 1.9531x over previous
"""Trainium2 Bass kernel for nn_BasicBlock (Minkowski sparse-conv block).

Single fused SPMD program on 8 cores, dest-sharded (core c owns output rows
[c*SHARD, (c+1)*SHARD)):
  AllGather x shards -> x_gat [8*PADSH, 128] bf16 in HBM
  conv: SWDGE dma_gather (transpose mode -> channel-major) per (d,k,s) run
        -> per-128-lane matmul vs W_k -> PSUM -> SBUF f32
        -> SWDGE dma_scatter_add into local y [ND*CH, 64] f32 (runtime counts)
  stats via ones-matmul + AllReduce, per-channel affine applied in flat
  tiles; h written bf16 padded-128, AllGather -> conv2 -> norm2 + residual
  + relu -> out shard bf16.

Host ships only: x shard bf16, compact wrapped idx tables (shared by both
convs), per-run counts, weights, gamma/beta.
"""
import numpy as np
import ml_dtypes

BF16 = ml_dtypes.bfloat16
EPS = 1e-5


class CFG:
    N, C = 400000, 64
    K, E = 27, 200000
    NCORES = 8
    SHARD = 50000
    PADSH = 50048              # padded shard rows (multiple of 128)
    CH = 32768                 # int16 index chunk
    SB_LANES = 8192            # lanes per superblock
    ROWT = 2048                # rows per flat norm tile

    @classmethod
    def derived(cls):
        cls.ND = (cls.SHARD + cls.CH - 1) // cls.CH
        cls.NS = (cls.NCORES * cls.PADSH + cls.CH - 1) // cls.CH
        cls.YROWS = cls.ND * cls.CH
        cls.GROWS = cls.NS * cls.CH
        cls.NT_FULL = cls.PADSH // cls.ROWT
        cls.TAILR = cls.PADSH - cls.NT_FULL * cls.ROWT
        assert cls.TAILR % 128 == 0 and cls.SHARD <= cls.PADSH
        assert cls.NCORES * cls.PADSH <= cls.GROWS


CFG.derived()

_cache = {}


def _route(in_idx, out_idx):
    """Host routing with per-run occurrence rounds (sub-runs).

    Within one dma_scatter_add the destination rows must be unique (the
    DMA's read-modify-write races otherwise), so each (d,k,s) run is split
    into sub-runs: sub-run r holds the r-th occurrence of each destination
    within that (core,run). Sub-runs become separate scatter instructions
    (tile serializes same-chunk scatters).

    Returns (runs, TOT, gidx, sidx, cnt_sub) where each run is
    (off, Ltot, d, k, s, [(sub_off, sub_L, cnt_col), ...]).
    """
    c = CFG
    ii = in_idx.reshape(-1).astype(np.int64)
    oo = out_idx.reshape(-1).astype(np.int64)
    M = ii.shape[0]
    kf = np.repeat(np.arange(c.K, dtype=np.int64), in_idx.shape[1])

    core = oo // c.SHARD
    dl = oo - core * c.SHARD
    d = dl // c.CH
    ld = dl - d * c.CH                       # scatter idx within chunk
    cs = ii // c.SHARD
    g = cs * c.PADSH + (ii - cs * c.SHARD)   # row in x_gat
    s = g // c.CH
    li = g - s * c.CH                        # gather idx within chunk

    NRALL = c.ND * c.K * c.NS
    rkey = (d * c.K + kf) * c.NS + s
    ckey = core * NRALL + rkey               # (core, run)

    # occurrence rank of each msg within (core, run, dst)
    okey = ckey * c.CH + ld
    oorder = np.argsort(okey, kind="stable")
    okey_s = okey[oorder]
    brk = np.r_[0, np.nonzero(np.diff(okey_s))[0] + 1]
    glen = np.diff(np.r_[brk, M])
    occ_s = np.arange(M, dtype=np.int64) - np.repeat(brk, glen)
    mult_s = np.repeat(glen, glen)
    occ = np.empty(M, np.int64)
    occ[oorder] = occ_s
    mult = np.empty(M, np.int64)
    mult[oorder] = mult_s

    # counts per (core, run, round)
    R = int(occ.max()) + 1
    cnt3 = np.bincount((ckey * R + occ).astype(np.int64),
                       minlength=c.NCORES * NRALL * R).reshape(
        c.NCORES, NRALL, R)
    cnt_r = cnt3.sum(0).astype(np.int64)          # total per (run, round)
    run_mask = cnt_r[:, 0] > 0
    maxr = (cnt_r > 0).argmin(1)                  # rounds per run
    maxr[cnt_r[:, -1] > 0] = R
    maxr[~run_mask] = 0

    # borrow: any (core, run, round<maxr) with 0 count gets one singleton
    # (mult==1, occ==0) lane moved into that round
    need = (cnt3 == 0) & (np.arange(R)[None, None, :] < maxr[None, :, None])
    if need.any():
        nc_, nr_, nro_ = np.nonzero(need)
        for cc, rr, ro in zip(nc_, nr_, nro_):
            cand = np.nonzero((ckey == cc * NRALL + rr) & (occ == 0)
                              & (mult == 1))[0]
            assert cand.size > 0, "no singleton to borrow"
            occ[cand[0]] = ro
            mult[cand[0]] = 0                     # not reusable
        cnt3 = np.bincount((ckey * R + occ).astype(np.int64),
                           minlength=c.NCORES * NRALL * R).reshape(
            c.NCORES, NRALL, R)

    # sub-run padded lengths and offsets
    Lsub = ((cnt3.max(0) + 127) // 128) * 128     # [NRALL, R]
    Ltot = Lsub.sum(1)
    run_ids = np.nonzero(Ltot)[0]
    roff = np.zeros(NRALL, np.int64)
    roff[run_ids] = np.cumsum(Ltot[run_ids]) - Ltot[run_ids]
    soff = np.cumsum(Lsub, 1) - Lsub              # sub offsets within run
    TOT = int(Ltot.sum())

    # lane position: sort by (core, run, round), rank within group
    skey = ckey * R + occ
    sorder = np.argsort(skey, kind="stable")
    skey_s = skey[sorder]
    sbrk = np.r_[0, np.nonzero(np.diff(skey_s))[0] + 1]
    sglen = np.diff(np.r_[sbrk, M])
    rank = np.arange(M, dtype=np.int64) - np.repeat(sbrk, sglen)
    rk_s = skey_s % (NRALL * R)
    run_s = rk_s // R
    rnd_s = rk_s % R
    lane = roff[run_s] + soff[run_s, rnd_s] + rank
    core_s = skey_s // (NRALL * R)

    gidx = np.zeros((c.NCORES, TOT), np.int16)
    sidx = np.full((c.NCORES, TOT), -1, np.int16)
    li_s = li[sorder]
    ld_s = ld[sorder]
    for cc in range(c.NCORES):
        m = core_s == cc
        gidx[cc, lane[m]] = li_s[m]
        sidx[cc, lane[m]] = ld_s[m]

    # per-core counts per sub-run (compacted column order)
    runs = []
    cols = []
    for r in run_ids:
        s_ = r % c.NS
        k_ = (r // c.NS) % c.K
        d_ = r // (c.NS * c.K)
        subs = []
        for ro in range(int(maxr[r])):
            subs.append((int(soff[r, ro]), int(Lsub[r, ro]), len(cols)))
            cols.append((r, ro))
        runs.append((int(roff[r]), int(Ltot[r]), int(d_), int(k_), int(s_),
                     subs))
    cnt_sub = np.stack([cnt3[:, r, ro] for (r, ro) in cols],
                       axis=1).astype(np.int32)
    assert (cnt_sub > 0).all(), "zero-count sub-run would hang scatter sem"
    return runs, TOT, gidx, sidx, cnt_sub


def _wrap16(a):
    """[..., n] -> [..., 16, n/16] wrapped: entry j at (j%16, j//16)."""
    n = a.shape[-1]
    assert n % 16 == 0
    if a.ndim == 2:
        return np.ascontiguousarray(a.reshape(a.shape[0], n // 16, 16)
                                    .transpose(0, 2, 1))
    return np.ascontiguousarray(a.reshape(n // 16, 16).T)


def _superblocks(runs):
    sbs, cur, acc = [], [], 0
    for r in runs:
        assert r[1] <= CFG.SB_LANES
        if acc + r[1] > CFG.SB_LANES and cur:
            sbs.append(cur)
            cur, acc = [], 0
        cur.append(r)
        acc += r[1]
    if cur:
        sbs.append(cur)
    return sbs


def _build_program(runs, TOT):
    from concourse import bacc, tile, mybir
    from concourse import library_config

    c = CFG
    F32 = mybir.dt.float32
    BF = mybir.dt.bfloat16
    I16 = mybir.dt.int16
    I32 = mybir.dt.int32
    ActF = mybir.ActivationFunctionType
    Alu = mybir.AluOpType

    NRUNS = sum(len(r[5]) for r in runs)     # scatter count columns
    sbs = _superblocks(runs)
    SBL = c.SB_LANES
    FTW = (c.ROWT // 128) * 64          # full row-tile width (1024)

    nc = bacc.Bacc("TRN2", target_bir_lowering=False, debug=False,
                   num_devices=c.NCORES)

    x_d = nc.dram_tensor("xs", [c.PADSH, 64], BF, kind="ExternalInput")
    gidx_d = nc.dram_tensor("gidx", [16, TOT // 16], I16,
                            kind="ExternalInput")
    sidx_d = nc.dram_tensor("sidx", [16, TOT // 16], I16,
                            kind="ExternalInput")
    cnt_d = nc.dram_tensor("cnt", [1, NRUNS], I32, kind="ExternalInput")
    w_d = nc.dram_tensor("wt", [64, 2 * c.K * 64], F32, kind="ExternalInput")
    gb_d = nc.dram_tensor("gb", [1, 4 * 64], F32, kind="ExternalInput")
    out_d = nc.dram_tensor("out", [c.PADSH, 64], BF, kind="ExternalOutput")

    with tile.TileContext(nc) as tc:
        with (
            tc.tile_pool(name="const", bufs=1) as constp,
            tc.tile_pool(name="gp", bufs=2) as gpool,
            tc.tile_pool(name="mp", bufs=2) as mpool,
            tc.tile_pool(name="ip", bufs=3) as ipool,
            tc.tile_pool(name="sp", bufs=2) as spool,
            tc.tile_pool(name="psmm", bufs=4, space="PSUM") as psmm,
            tc.tile_pool(name="psbc", bufs=1, space="PSUM") as psbc,
            tc.tile_pool(name="psst", bufs=1, space="PSUM") as psst,
            tc.tile_pool(name="dram", bufs=1, space="DRAM") as dramp,
        ):
            nc.gpsimd.load_library(library_config.mlp)

            # ---------- constants
            w_f = constp.tile([64, 2 * c.K * 64], F32)
            nc.sync.dma_start(w_f[:], w_d[:])
            w_b = constp.tile([64, 2 * c.K * 64], BF)
            nc.vector.tensor_copy(w_b[:], w_f[:])
            gb_t = constp.tile([1, 4 * 64], F32)
            nc.sync.dma_start(gb_t[:], gb_d[:])
            ones_col = constp.tile([128, 1], F32)
            nc.vector.memset(ones_col[:], 1.0)
            ones_row = constp.tile([1, 128], F32)
            nc.vector.memset(ones_row[:], 1.0)
            epst = constp.tile([1, 1], F32)
            nc.vector.memset(epst[:], EPS)
            zt = constp.tile([128, 4096], F32)
            nc.vector.memset(zt[:], 0.0)

            cnt_t = constp.tile([1, NRUNS], I32)
            nc.sync.dma_start(cnt_t[:], cnt_d[:])

            # ---------- DRAM scratch
            NPG = c.NCORES * c.PADSH
            x_gat = dramp.tile([c.GROWS, 128], BF)
            h_loc = dramp.tile([c.PADSH, 64], BF)
            h_gat = dramp.tile([c.GROWS, 128], BF)
            x64_st = dramp.tile([c.PADSH, 64], BF)
            xg64 = nc.dram_tensor("xg64", [NPG, 64], BF, kind="Internal",
                                  addr_space="Shared")
            hg64 = nc.dram_tensor("hg64", [NPG, 64], BF, kind="Internal",
                                  addr_space="Shared")
            y1 = dramp.tile([c.YROWS, 64], F32)
            y2 = dramp.tile([c.YROWS, 64], F32)
            st_in = dramp.tile([1, 128], F32)
            st_out = dramp.tile([1, 128], F32)

            def pad_copy(dst128, src64):
                """[NPG,64] contiguous -> [:,0:64] of [GROWS,128] strided."""
                step = 8192
                for r0 in range(0, NPG, step):
                    r1 = min(NPG, r0 + step)
                    nc.sync.dma_start(dst128[r0:r1, 0:64], src64[r0:r1, :])

            for yb in (y1, y2):
                yv = yb[:].rearrange("(p b) c -> p (b c)", p=128)
                wv = yv.shape[1]
                for j in range(0, wv, 4096):
                    zw = min(4096, wv - j)
                    nc.sync.dma_start(yv[:, j:j + zw], zt[:, 0:zw])

            nc.sync.dma_start(x64_st[:], x_d[:])
            nc.gpsimd.collective_compute(
                "AllGather", Alu.bypass,
                replica_groups=[list(range(c.NCORES))],
                ins=[x64_st[:]], outs=[xg64[:]],
            )
            pad_copy(x_gat, xg64)

            creg = nc.gpsimd.alloc_register("scnt")

            # ---------- sparse conv
            def issue_gathers(sb, src_gat):
                lanes = sum(r[1] for r in sb)
                base = sb[0][0]
                gi_t = ipool.tile([128, SBL // 16], I16, tag="gi")
                si_t = ipool.tile([128, SBL // 16], I16, tag="si")
                for (t, d_src) in ((gi_t, gidx_d), (si_t, sidx_d)):
                    nc.sync.dma_start(
                        t[0:16, 0:lanes // 16],
                        d_src[:, base // 16:(base + lanes) // 16])
                    nc.sync.dma_start(t[16:32, 0:lanes // 16],
                                      t[0:16, 0:lanes // 16])
                    nc.sync.dma_start(t[32:64, 0:lanes // 16],
                                      t[0:32, 0:lanes // 16])
                    nc.sync.dma_start(t[64:128, 0:lanes // 16],
                                      t[0:64, 0:lanes // 16])
                GCAP = 896   # transpose dma_gather hangs at >=1024 idxs
                g_t = gpool.tile([128, SBL], BF, tag="g")
                for (off, L, d_, k_, s_, subs) in sb:
                    lo = off - base
                    for p0 in range(0, L, GCAP):
                        pL = min(GCAP, L - p0)
                        a = lo + p0
                        nc.gpsimd.dma_gather(
                            out_ap=g_t[:, a:a + pL].unsqueeze(1),
                            in_ap=src_gat[s_ * c.CH:(s_ + 1) * c.CH, :],
                            idxs_ap=gi_t[:, a // 16:(a + pL) // 16],
                            num_idxs=pL,
                            num_idxs_reg=pL,
                            elem_size=128,
                            transpose=True,
                        )
                return g_t, si_t

            def compute_and_scatter(sb, g_t, si_t, y_dst, kofs):
                lanes = sum(r[1] for r in sb)
                nblk = lanes // 128
                base = sb[0][0]
                msg_t = mpool.tile([128, (SBL // 128) * 64], F32, tag="msg")
                bk = []
                for (off, L, d_, k_, s_, subs) in sb:
                    bk += [k_] * (L // 128)
                for g0 in range(0, nblk, 8):
                    g1 = min(nblk, g0 + 8)
                    mm = psmm.tile([128, 512], F32, tag="mm")
                    for b in range(g0, g1):
                        nc.tensor.matmul(
                            out=mm[:, (b - g0) * 64:(b - g0 + 1) * 64],
                            lhsT=g_t[0:64, b * 128:(b + 1) * 128],
                            rhs=w_b[:, (kofs + bk[b]) * 64:
                                    (kofs + bk[b] + 1) * 64],
                            start=True, stop=True,
                        )
                    dst = msg_t[:, g0 * 64:g1 * 64]
                    src = mm[:, 0:(g1 - g0) * 64]
                    if (g0 // 8) % 2 == 0:
                        nc.scalar.activation(dst, src, ActF.Copy)
                    else:
                        nc.vector.tensor_copy(dst, src)
                for (off, L, d_, k_, s_, subs) in sb:
                    lo = off - base
                    for (so, sL, col) in subs:
                        a = lo + so
                        nc.gpsimd.reg_load(creg, cnt_t[0:1, col:col + 1])
                        nc.gpsimd.dma_scatter_add(
                            out_ap=y_dst[d_ * c.CH:(d_ + 1) * c.CH, :],
                            in_ap=msg_t[:, a // 128 * 64:(a + sL) // 128 * 64]
                            .rearrange("p (b ch) -> p b ch", ch=64),
                            idxs_ap=si_t[:, a // 16:(a + sL) // 16],
                            num_idxs=sL,
                            num_idxs_reg=creg,
                            elem_size=64,
                        )

            def conv(src_gat, y_dst, kofs):
                stage = []
                for sb in sbs:
                    g_t, si_t = issue_gathers(sb, src_gat)
                    stage.append((sb, g_t, si_t))
                    if len(stage) >= 2:
                        psb, pg, psi = stage.pop(0)
                        compute_and_scatter(psb, pg, psi, y_dst, kofs)
                while stage:
                    psb, pg, psi = stage.pop(0)
                    compute_and_scatter(psb, pg, psi, y_dst, kofs)

            def row_tiles():
                out = []
                for t in range(c.NT_FULL + 1):
                    r0 = t * c.ROWT
                    nr = c.ROWT if t < c.NT_FULL else c.TAILR
                    if nr:
                        out.append((t, r0, nr, nr // 128))
                return out

            def load_rowtile(y_src, r0, nr, w, tag):
                yv = spool.tile([128, FTW], F32, tag=tag)
                if w < FTW:
                    nc.vector.memset(yv[:, 0:FTW], 0.0)
                nc.sync.dma_start(
                    yv[:, 0:w],
                    y_src[r0:r0 + nr, :].rearrange("(p b) ch -> p (b ch)",
                                                   p=128))
                return yv

            SW = min(512, FTW)          # stats psum width

            def stats_affine(y_src):
                """Channel sums + sumsq over the shard, AllReduced."""
                tiles = row_tiles()
                ssum = psst.tile([1, SW], F32, tag="ssum")
                ssq = psst.tile([1, SW], F32, tag="ssq")
                nchunk = len(tiles) * (FTW // SW)
                qi = 0
                for t, r0, nr, nb in tiles:
                    w = nb * 64
                    yv = load_rowtile(y_src, r0, nr, w, "yv")
                    sq = spool.tile([128, FTW], F32, tag="sq")
                    nc.scalar.activation(sq[:, 0:w], yv[:, 0:w], ActF.Square)
                    if w < FTW:
                        nc.vector.memset(sq[:, w:FTW], 0.0)
                    for c0 in range(0, FTW, SW):
                        nc.tensor.matmul(out=ssum[:], lhsT=ones_col[:],
                                         rhs=yv[:, c0:c0 + SW],
                                         start=(qi == 0),
                                         stop=(qi == nchunk - 1))
                        nc.tensor.matmul(out=ssq[:], lhsT=ones_col[:],
                                         rhs=sq[:, c0:c0 + SW],
                                         start=(qi == 0),
                                         stop=(qi == nchunk - 1))
                        qi += 1
                # fold SW columns down to 64, pack [sum, sumsq] into [1,128]
                fold = spool.tile([1, 2 * SW], F32, tag="fold")
                nc.vector.tensor_copy(fold[:, 0:SW], ssum[:])
                nc.vector.tensor_copy(fold[:, SW:2 * SW], ssq[:])
                for half in range(2):
                    b0 = half * SW
                    step = SW // 2
                    while step >= 64:
                        nc.vector.tensor_tensor(
                            out=fold[:, b0:b0 + step],
                            in0=fold[:, b0:b0 + step],
                            in1=fold[:, b0 + step:b0 + 2 * step],
                            op=Alu.add)
                        step //= 2
                packed = spool.tile([1, 128], F32, tag="packed")
                nc.vector.tensor_copy(packed[:, 0:64], fold[:, 0:64])
                nc.vector.tensor_copy(packed[:, 64:128], fold[:, SW:SW + 64])
                nc.sync.dma_start(st_in[:], packed[:])
                nc.gpsimd.collective_compute(
                    "AllReduce", Alu.add,
                    replica_groups=[list(range(c.NCORES))],
                    ins=[st_in[:]], outs=[st_out[:]],
                )
                allst = spool.tile([1, 128], F32, tag="allst")
                nc.sync.dma_start(allst[:], st_out[:])
                return allst

            def affine_consts(allst, gofs):
                """a = gamma*rsqrt(var+eps), b = beta - mu*a; [128,FTW] reps."""
                invN = 1.0 / float(c.N)
                mu = spool.tile([1, 64], F32, tag="mu")
                nc.vector.tensor_scalar(out=mu[:], in0=allst[0:1, 0:64],
                                        scalar1=invN, scalar2=None,
                                        op0=Alu.mult)
                ex2 = spool.tile([1, 64], F32, tag="ex2")
                nc.vector.tensor_scalar(out=ex2[:], in0=allst[0:1, 64:128],
                                        scalar1=invN, scalar2=None,
                                        op0=Alu.mult)
                var = spool.tile([1, 64], F32, tag="var")
                nc.vector.tensor_tensor(out=var[:], in0=mu[:], in1=mu[:],
                                        op=Alu.mult)
                nc.vector.tensor_tensor(out=var[:], in0=ex2[:], in1=var[:],
                                        op=Alu.subtract)
                nc.vector.tensor_scalar(out=var[:], in0=var[:],
                                        scalar1=epst[0:1, 0:1], scalar2=None,
                                        op0=Alu.add)
                sd = spool.tile([1, 64], F32, tag="sd")
                nc.scalar.activation(sd[:], var[:], ActF.Sqrt)
                rstd = spool.tile([1, 64], F32, tag="rstd")
                nc.vector.reciprocal(rstd[:], sd[:])
                a_c = spool.tile([1, 64], F32, tag="a_c")
                nc.vector.tensor_tensor(out=a_c[:], in0=rstd[:],
                                        in1=gb_t[0:1, gofs:gofs + 64],
                                        op=Alu.mult)
                b_c = spool.tile([1, 64], F32, tag="b_c")
                nc.vector.tensor_tensor(out=b_c[:], in0=mu[:], in1=a_c[:],
                                        op=Alu.mult)
                nc.vector.tensor_tensor(out=b_c[:],
                                        in0=gb_t[0:1, gofs + 64:gofs + 128],
                                        in1=b_c[:], op=Alu.subtract)
                # broadcast to 128 partitions, tile 16x along free
                reps = []
                for src in (a_c, b_c):
                    bc = psbc.tile([128, 64], F32, tag="bc")
                    nc.tensor.matmul(out=bc[:], lhsT=ones_row[:], rhs=src[:],
                                     start=True, stop=True)
                    rep = spool.tile([128, FTW], F32, tag=f"rep{len(reps)}")
                    nc.scalar.activation(rep[:, 0:64], bc[:], ActF.Copy)
                    width = 64
                    while width < FTW:
                        wnext = min(FTW, 2 * width)
                        nc.vector.tensor_copy(rep[:, width:wnext],
                                              rep[:, 0:wnext - width])
                        width = wnext
                    reps.append(rep)
                return reps

            def apply_norm(y_src, a_rep, b_rep, mode):
                """mode 'h': h_loc = relu(a*y+b) bf16 (cols 0:64).
                   mode 'out': out_d = relu(a*y+b + x) bf16."""
                for t, r0, nr, nb in row_tiles():
                    w = nb * 64
                    yv = load_rowtile(y_src, r0, nr, w, "ya")
                    nc.vector.tensor_tensor(out=yv[:, 0:w], in0=yv[:, 0:w],
                                            in1=a_rep[:, 0:w], op=Alu.mult)
                    nc.vector.tensor_tensor(out=yv[:, 0:w], in0=yv[:, 0:w],
                                            in1=b_rep[:, 0:w], op=Alu.add)
                    if mode == "out":
                        xb = spool.tile([128, FTW], BF, tag="xb")
                        nc.sync.dma_start(
                            xb[:, 0:w],
                            x_d[r0:r0 + nr, :].rearrange(
                                "(p b) ch -> p (b ch)", p=128))
                        xf = spool.tile([128, FTW], F32, tag="xf")
                        nc.scalar.activation(xf[:, 0:w], xb[:, 0:w],
                                             ActF.Copy)
                        nc.vector.tensor_tensor(out=yv[:, 0:w],
                                                in0=yv[:, 0:w],
                                                in1=xf[:, 0:w], op=Alu.add)
                    ob = spool.tile([128, FTW], BF, tag="ob")
                    nc.scalar.activation(ob[:, 0:w], yv[:, 0:w], ActF.Relu)
                    if mode == "h":
                        nc.sync.dma_start(
                            h_loc[r0:r0 + nr, :].rearrange(
                                "(p b) ch -> p (b ch)", p=128),
                            ob[:, 0:w])
                    else:
                        nc.sync.dma_start(
                            out_d[r0:r0 + nr, :].rearrange(
                                "(p b) ch -> p (b ch)", p=128),
                            ob[:, 0:w])

            # ---------------- pipeline
            conv(x_gat, y1, kofs=0)
            allst1 = stats_affine(y1)
            a1, b1 = affine_consts(allst1, gofs=0)
            apply_norm(y1, a1, b1, "h")
            nc.gpsimd.collective_compute(
                "AllGather", Alu.bypass,
                replica_groups=[list(range(c.NCORES))],
                ins=[h_loc[:]], outs=[hg64[:]],
            )
            pad_copy(h_gat, hg64)
            conv(h_gat, y2, kofs=c.K)
            allst2 = stats_affine(y2)
            a2, b2 = affine_consts(allst2, gofs=128)
            apply_norm(y2, a2, b2, "out")

    nc.compile()
    return nc


def kernel(x, in_idx, out_idx, W1, W2, gamma1, beta1, gamma2, beta2,
           profile=False):
    from concourse.bass_utils import run_bass_kernel_spmd
    import time as _t

    c = CFG
    x = np.asarray(x, np.float32)
    W1 = np.asarray(W1, np.float32)
    W2 = np.asarray(W2, np.float32)
    g1 = np.asarray(gamma1, np.float32)
    b1 = np.asarray(beta1, np.float32)
    g2 = np.asarray(gamma2, np.float32)
    b2 = np.asarray(beta2, np.float32)

    runs, TOT, gidx, sidx, cnt_sub = _route(np.asarray(in_idx),
                                            np.asarray(out_idx))

    key = (TOT, tuple((r[0], r[1], tuple(r[5])) for r in runs))
    if key not in _cache:
        _cache[key] = _build_program(runs, TOT)
    nc = _cache[key]

    # W layout: [64 cin, (conv,k) * 64 cout]
    wt = np.concatenate([W1, W2], axis=0).transpose(1, 0, 2).reshape(
        64, 2 * c.K * 64)
    wt = np.ascontiguousarray(wt)
    gb = np.concatenate([g1, b1, g2, b2])[None, :].astype(np.float32)

    in_maps = []
    for cc in range(c.NCORES):
        xs = np.zeros((c.PADSH, 64), BF16)
        xs[0:c.SHARD] = x[cc * c.SHARD:(cc + 1) * c.SHARD]
        in_maps.append({
            "xs": xs,
            "gidx": _wrap16(gidx[cc]),
            "sidx": _wrap16(sidx[cc]),
            "cnt": cnt_sub[cc][None, :],
            "wt": wt,
            "gb": gb,
        })

    t0 = _t.time()
    try:
        results = _run_spmd_cached(nc, in_maps)
    except Exception:
        res = run_bass_kernel_spmd(nc, in_maps,
                                   core_ids=list(range(c.NCORES)),
                                   trace=profile)
        results = res.results
        kernel._last = res
    kernel._run_s = _t.time() - t0

    out = np.empty((c.N, c.C), np.float32)
    for cc in range(c.NCORES):
        out[cc * c.SHARD:(cc + 1) * c.SHARD] = \
            results[cc]["out"][0:c.SHARD].astype(np.float32)
    return out


_jit_cache = {}


def _run_spmd_cached(nc, in_maps):
    """run_bass_via_pjrt with the jitted executable cached across calls."""
    import jax
    import concourse.mybir as mybir
    from concourse import bass2jax
    from concourse.bass2jax import (_bass_exec_p, install_neuronx_cc_hook,
                                    partition_id_tensor)
    from jax.sharding import Mesh, PartitionSpec
    from jax.experimental.shard_map import shard_map

    n_cores = CFG.NCORES
    key = id(nc)
    if key not in _jit_cache:
        install_neuronx_cc_hook()
        assert nc.dbg_addr is None
        partition_name = (nc.partition_id_tensor.name
                          if nc.partition_id_tensor else None)
        in_names, out_names, out_avals = [], [], []
        for alloc in nc.m.functions[0].allocations:
            if not isinstance(alloc, mybir.MemoryLocationSet):
                continue
            name = alloc.memorylocations[0].name
            if alloc.kind == "ExternalInput":
                if name != partition_name:
                    in_names.append(name)
            elif alloc.kind == "ExternalOutput":
                out_names.append(name)
                out_avals.append(jax.core.ShapedArray(
                    tuple(alloc.tensor_shape), mybir.dt.np(alloc.dtype)))
        n_params = len(in_names)
        all_in = in_names + out_names
        if partition_name is not None:
            all_in.append(partition_name)
        donate = tuple(range(n_params, n_params + len(out_names)))

        def _body(*args):
            operands = list(args)
            if partition_name is not None:
                operands.append(partition_id_tensor())
            return tuple(_bass_exec_p.bind(
                *operands,
                out_avals=tuple(out_avals),
                in_names=tuple(all_in),
                out_names=tuple(out_names),
                lowering_input_output_aliases=(),
                sim_require_finite=True,
                sim_require_nnan=True,
                nc=nc,
            ))

        devices = jax.devices()[:n_cores]
        mesh = Mesh(np.asarray(devices), ("core",))
        nio = n_params + len(out_names)
        fn = jax.jit(
            shard_map(_body, mesh=mesh,
                      in_specs=(PartitionSpec("core"),) * nio,
                      out_specs=(PartitionSpec("core"),) * len(out_names),
                      check_rep=False),
            donate_argnums=donate, keep_unused=True)
        _jit_cache[key] = (fn, in_names, out_names, out_avals, n_params)

    fn, in_names, out_names, out_avals, n_params = _jit_cache[key]
    concat_in = [
        np.concatenate([np.asarray(in_maps[cc][nm]) for cc in range(n_cores)],
                       axis=0)
        for nm in in_names
    ]
    concat_zeros = [
        np.zeros((n_cores * av.shape[0], *av.shape[1:]), av.dtype)
        for av in out_avals
    ]
    out_arrs = fn(*concat_in, *concat_zeros)
    return [
        {nm: np.asarray(out_arrs[i]).reshape(n_cores, *out_avals[i].shape)[cc]
         for i, nm in enumerate(out_names)}
        for cc in range(n_cores)
    ]
